# revision 16
# baseline (speedup 1.0000x reference)
"""Trainium2 Bass kernel for nn_LocalitySelfAttention.

The module's attention scores get +1e9 added on the diagonal before the
softmax (torch's ``attn - diag(-1e9)``).  QK^T scores for randn inputs are
O(1), so every softmax row is an exact fp32 one-hot at the diagonal and
``attn @ v == v`` bit-exactly.  The whole module therefore reduces to

    out = x @ Wv.T @ w_proj.T + b_proj,      Wv = w_qkv[512:768]

which is a memory-bound GEMM.  The kernel shards the 8192 (B*N) rows across
the 8 NeuronCores (1024 rows each).

Measured HW model (from perfetto/NTFF analysis):
  - exec_time = last-useful-instr end - first-useful start.  The NRT
    postamble (each engine serially zeroing ~51 semaphores; Tensor is the
    straggler at ~144ns each) plus exit barriers is a ~8.5us constant tail
    AFTER the last output-DMA completion semaphore, so everything aims at
    finishing the last output byte early.
  - The measured window opens at the framework const memsets (~6.0us);
    first DMA trigger ~6.7us; first bytes land ~1.5us later.
  - Stream shape is delicate: Sync-only schedules (whether 6x2KB or
    3x4KB triggers) measure 1.5-2.5us SLOWER end-to-end than the mixed
    dual-ring layout, so the baseline mix is kept: wb as one 4KB-line
    DMA then the middle x half as 2KB-line kc planes on the SP ring; the
    first and last x quarters as 1KB-line kc planes on the Act ring.
    Completion semaphores fire when the SLOWEST queue passes that DMA's
    descriptors, so accumulated queue skew makes mid/late chunk
    semaphores bunch 1-2.5us after their data regardless of chunking.
  - PSUM-reading ops are the drain bottleneck: DVE ops touching PSUM
    take ~425ns per [128,256] tile (SBUF-only: 292ns), Act ACTIVATE
    ~474ns; GpSimd cannot read PSUM.  The 8 tile drains alternate
    DVE / Act.  Act cannot add a free-dim bias, so Act-drained tiles
    get the bias pre-loaded into PSUM by an f32r E-row matmul
    (stationary row 0 = ones, moving row 0 = bias) at standard matmul
    cost; plain-f32 operands would dual-pass at ~2x.
  - The PE clock starts at a low pstate (~213-500ns per 128-row f32r
    matmul) and ramps to ~112ns only after ~4.8us of sustained matmul
    activity; idle gaps stall the credit, so warmup bursts fill every
    semaphore-wait gap (pre-fold, post-fold, and between tile groups).

Schedule:
  - SP ring: wb (4KB lines) -> bias (1 descriptor) -> x cols 256-767 as
    two 2KB-line kc planes.  Act ring: x cols 0-255 then 768-1023, each
    as two 1KB-line kc planes.  Tiles 0-1 unlock on the first Act
    quarters (~11.9us), tiles 2-5 on the SP half, tiles 6-7 on the last
    Act quarters.
  - Fold W2T = Wv @ WprojT on the wb semaphore, f32r end-to-end.
  - Outputs: 2-tile DMAs alternating rings as their tiles drain; tile 6
    single; tile 7 split into two 64-partition DMAs, one per ring (half
    the trigger cost on the critical tail).

The host only moves bytes: it transposes/packs x and the weights and
unpermutes/widens the per-core output blocks (layout + zero-extension
only, no arithmetic).
"""

import os
import sys

import numpy as np

if "/opt/trn_rl_repo" not in sys.path:
    sys.path.insert(0, "/opt/trn_rl_repo")

B, N, C = 2, 4096, 256
ROWS = B * N              # 8192
NCORES = 8
RPC = ROWS // NCORES      # 1024 rows per core
NT = RPC // 128           # 8 row-tiles of 128 per core

NWARM = int(os.environ.get("K_NWARM", "6"))    # PE ramp pairs, pre-fold
NWARM2 = int(os.environ.get("K_NWARM2", "2"))  # post-fold filler pairs
NWARM3 = int(os.environ.get("K_NWARM3", "6"))  # tile-gap filler pairs
ACTDRAIN = os.environ.get("K_ACTDRAIN", "1") == "1"  # odd tiles via Act

# x column groups: (engine_name, col0, ncols, line_bytes)
#   tiles 0-1 <- Act quarter (early, its sems land first)
#   tiles 2-5 <- SP half
#   tiles 6-7 <- Act quarter (late)
GROUPS = [("scalar", 0, 256), ("sync", 256, 512), ("scalar", 768, 256)]
TILE_GRP = [0, 0, 1, 1, 1, 1, 2, 2]

_cache = {}


def _build():
    """Build + compile the per-core Bass program (same program, SPMD)."""
    import concourse.bacc as bacc
    import concourse.bass as bass
    import concourse.mybir as mybir
    import concourse.tile as tile

    f32 = mybir.dt.float32
    mm_dt = mybir.dt.float32r
    out_dt = mybir.dt.bfloat16

    nc = bacc.Bacc(
        "TRN2",
        target_bir_lowering=False,
        debug=False,
        num_devices=NCORES,
    )

    # All matmul inputs are typed f32r in DRAM too (bytes are plain fp32).
    xt_d = nc.dram_tensor("xt", [C, RPC], mm_dt, kind="ExternalInput")
    # wb[p, 0]=Wv[p], [p,1]=WprojT[p], [p,2]=Wv[128+p], [p,3]=WprojT[128+p]
    wb_d = nc.dram_tensor("wb", [128, 4, C], mm_dt, kind="ExternalInput")
    b_d = nc.dram_tensor("b", [1, C], f32, kind="ExternalInput")
    # output laid out [p, t, m]; the host undoes the (t p) permutation
    out_d = nc.dram_tensor("out", [128, NT * C], out_dt, kind="ExternalOutput")

    xt = xt_d.ap()
    wb = wb_d.ap()
    b = b_d.ap()
    out = out_d.ap()

    with tile.TileContext(nc) as tc:
        with (
            tc.tile_pool(name="const", bufs=1) as cp,
            tc.tile_pool(name="psw", bufs=3, space="PSUM") as psw,
            tc.tile_pool(name="pso", bufs=5, space="PSUM") as pso,
        ):
            # ---- SP ring: weights (4KB lines) then bias (1 descriptor) ----
            wb_sb = cp.tile([128, 4, C], mm_dt)
            nc.sync.dma_start(out=wb_sb, in_=wb)
            bias_sb = cp.tile([1, C], f32)
            nc.sync.dma_start(out=bias_sb, in_=b)

            # ---- x chunks: kc planes per group ----
            xt_v = xt.rearrange("(kc p) n -> p kc n", p=128)
            engs = {"sync": nc.sync, "scalar": nc.scalar}
            xt_sbs = []      # [group][kc] -> tile [128, 1, ncols]
            for gi, (ename, c0, clen) in enumerate(GROUPS):
                pair = []
                for kc in range(2):
                    xs = cp.tile([128, 1, clen], mm_dt,
                                 name=f"xchunk{gi}_{kc}", tag=f"xchunk{gi}_{kc}")
                    engs[ename].dma_start(
                        out=xs, in_=xt_v[:, kc:kc + 1, c0:c0 + clen])
                    pair.append(xs)
                xt_sbs.append(pair)

            ones_sb = cp.tile([1, 128], f32)
            nc.vector.memset(ones_sb, 1.0)

            # ---- PE warmup: ends near the wb semaphore ----
            warm_sb = cp.tile([128, 128], f32)
            nc.vector.memset(warm_sb, 0.0)
            warm_ps = psw.tile([128, C], f32, tag="w")

            def warm_burst(n):
                for _ in range(n):
                    nc.tensor.matmul(
                        warm_ps[:, 0:128], warm_sb, warm_sb,
                        start=True, stop=True,
                    )

            warm_burst(NWARM)

            # ---- fold W2T[k, p] = sum_vd Wv[vd, k] * WprojT[vd, p] ----
            w2t_sb = cp.tile([128, 2, C], mm_dt)  # [p(k), kc, pcol]
            for kc in range(2):
                ps = psw.tile([128, C], f32, tag="w")
                for vdc in range(2):
                    nc.tensor.matmul(
                        ps,
                        wb_sb[:, 2 * vdc, kc * 128:(kc + 1) * 128],
                        wb_sb[:, 2 * vdc + 1, :],
                        start=(vdc == 0),
                        stop=(vdc == 1),
                    )
                nc.vector.tensor_copy(w2t_sb[:, kc, :], ps)

            warm_burst(NWARM2)

            # bias row block for the DVE drains (PE broadcast)
            bias_bc = cp.tile([128, C], f32)
            ps_b = psw.tile([128, C], f32, tag="w")
            nc.tensor.matmul(ps_b, ones_sb, bias_sb, start=True, stop=True)
            nc.vector.tensor_copy(bias_bc, ps_b)

            if ACTDRAIN:
                # Act-drained tiles: pure PSUM->SBUF bf16 cast on Act,
                # then a cheap bf16 DVE add of the (bf16) bias block --
                # 16-bit DVE ops run ~2x the f32 rate and the extra
                # rounding is far inside the 2e-2 tolerance
                biasbf = cp.tile([128, C], out_dt)
                nc.vector.tensor_copy(biasbf, bias_bc)
                otr_sb = cp.tile([128, NT, C], out_dt, name="otr")

            # big ramp filler: runs during the x semaphore waits so the
            # PE's DVFS credit is earned before the tile burst
            warm_burst(NWARM3)

            # ---- main GEMM: out[n, p] = b[p] + sum_k xT[k, n]*W2T[k, p] ----
            ot_sb = cp.tile([128, NT, C], out_dt)
            for t in range(NT):
                gi = TILE_GRP[t]
                off = t * 128 - GROUPS[gi][1]
                xk0, xk1 = xt_sbs[gi]
                ps = pso.tile([128, C], f32)
                actdrain = ACTDRAIN and (t % 2 == 1)
                nc.tensor.matmul(
                    ps, xk0[:, 0, off:off + 128], w2t_sb[:, 0, :],
                    start=True, stop=False,
                )
                nc.tensor.matmul(
                    ps, xk1[:, 0, off:off + 128], w2t_sb[:, 1, :],
                    start=False, stop=True,
                )
                if actdrain:
                    # Act casts PSUM->SBUF; the idle GpSimd does the bf16
                    # bias add SBUF->SBUF, keeping the DVE queue free of
                    # cross-engine head-of-line stalls
                    nc.scalar.copy(otr_sb[:, t, :], ps)
                    nc.gpsimd.tensor_add(ot_sb[:, t, :], otr_sb[:, t, :],
                                         biasbf)
                else:
                    nc.vector.tensor_add(ot_sb[:, t, :], ps, bias_bc)

                # two fat output DMAs: the first half early on SP; the
                # second half at the tail, split across both rings by
                # partition halves (parallel half-price triggers)
                if t == 3:
                    nc.sync.dma_start(out=out[:, 0:4 * C],
                                      in_=ot_sb[:, 0:4, :])
                elif t == 7:
                    nc.sync.dma_start(out=out[0:64, 4 * C:8 * C],
                                      in_=ot_sb[0:64, 4:8, :])
                    nc.scalar.dma_start(out=out[64:128, 4 * C:8 * C],
                                        in_=ot_sb[64:128, 4:8, :])

    nc.compile()
    return nc


def _pack_inputs(x, w_qkv, w_proj, b_proj):
    """Host-side layout marshaling only (no FLOPs)."""
    xT = np.ascontiguousarray(x.reshape(ROWS, C).T)          # [256, 8192]
    wv = w_qkv[2 * C:3 * C]                                  # [256, 256]
    wpt = w_proj.T                                           # [256, 256]
    wb = np.empty((128, 4, C), dtype=np.float32)
    wb[:, 0] = wv[0:128]
    wb[:, 1] = wpt[0:128]
    wb[:, 2] = wv[128:256]
    wb[:, 3] = wpt[128:256]
    wb = np.ascontiguousarray(wb)
    b2 = np.ascontiguousarray(b_proj.reshape(1, C))

    in_maps = [
        {
            "xt": np.ascontiguousarray(xT[:, c * RPC:(c + 1) * RPC]),
            "wb": wb,
            "b": b2,
        }
        for c in range(NCORES)
    ]
    return in_maps


def run_sharded(inputs, trace=False, trace_cores=None):
    """Shard inputs, run on the 8 NeuronCores, gather.  Returns
    (full_output, BassKernelResults)."""
    from concourse.bass_utils import run_bass_kernel_spmd

    x = np.ascontiguousarray(np.asarray(inputs["x"], dtype=np.float32))
    w_qkv = np.ascontiguousarray(np.asarray(inputs["w_qkv"], dtype=np.float32))
    w_proj = np.ascontiguousarray(np.asarray(inputs["w_proj"], dtype=np.float32))
    b_proj = np.ascontiguousarray(np.asarray(inputs["b_proj"], dtype=np.float32))

    if "nc" not in _cache:
        _cache["nc"] = _build()
    nc = _cache["nc"]

    in_maps = _pack_inputs(x, w_qkv, w_proj, b_proj)

    res = run_bass_kernel_spmd(
        nc,
        in_maps,
        core_ids=list(range(NCORES)),
        trace=trace,
        trace_cores=trace_cores,
    )
    # device emits [p, t, m]; undo the (t p) row permutation and widen
    # bf16 -> f32 (exact zero-extension)
    blocks = []
    for c in range(NCORES):
        arr = np.asarray(res.results[c]["out"]).reshape(128, NT, C)
        blocks.append(
            np.ascontiguousarray(arr.transpose(1, 0, 2)).reshape(RPC, C).astype(np.float32)
        )
    out = np.concatenate(blocks, axis=0)  # [8192, 256]
    return out.reshape(B, N, C), res


def kernel(x, w_qkv, w_proj, b_proj, temperature):
    out, _ = run_sharded(
        {"x": x, "w_qkv": w_qkv, "w_proj": w_proj, "b_proj": b_proj}
    )
    return out


# revision 17
# speedup vs baseline: 1.0759x; 1.0759x over previous
"""Trainium2 Bass kernel for nn_LocalitySelfAttention.

The module's attention scores get +1e9 added on the diagonal before the
softmax (torch's ``attn - diag(-1e9)``).  QK^T scores for randn inputs are
O(1), so every softmax row is an exact fp32 one-hot at the diagonal and
``attn @ v == v`` bit-exactly.  The whole module therefore reduces to

    out = x @ Wv.T @ w_proj.T + b_proj,      Wv = w_qkv[512:768]

which is a memory-bound GEMM.  The kernel shards the 8192 (B*N) rows across
the 8 NeuronCores (1024 rows each).

Measured HW model (from perfetto/NTFF analysis):
  - exec_time = last-useful-instr end - first-useful start.  The NRT
    postamble (each engine serially zeroing ~51 semaphores; Tensor is the
    straggler at ~144ns each) plus exit barriers is a ~8.5us constant
    tail AFTER the last output-DMA completion semaphore, so everything
    aims at finishing the last output byte early.
  - The measured window opens at the framework const memsets (~6.0us);
    first DMA trigger ~6.7us; first bytes land ~1.5us later; the stream
    then runs at ~260-360 GB/s.
  - Completion semaphores fire when the SLOWEST of the 16 HW queues
    passes that DMA's descriptors.  Queue skew accumulates over the
    stream (HBM contention with the other 7 cores makes it stochastic),
    so mid/late chunk semaphores bunch 1-2.5us after their data lands
    no matter how the chunks are cut.  Single-ring schedules starve
    (~700ns per 128-descriptor trigger, serialized); three-DMA 4KB-line
    schedules measured SLOWER end to end, so inputs stay split across
    both HWDGE rings with one mid-priority chunk on Act (whose ~2.5us
    cold start is hidden by data not needed until mid-stream).
  - PSUM-reading ops: DVE ~425ns per [128,256] tile, Act ACTIVATE ~474;
    GpSimd cannot read PSUM; SWDGE (gpsimd dma) has ~4us latency - dead
    end for outputs.
  - The PE clock starts at a low pstate (~213-500ns per 128-row f32r
    matmul) and ramps to ~112ns only after ~4.8us of sustained matmul
    activity; idle gaps stall the credit, so warmup bursts bracket the
    fold.

Schedule:
  - Host packs x^T per-core as 4 column chunks of 256, each partition
    line [kc0 256 | kc1 256] contiguous (2KB lines): one DMA and ONE
    completion semaphore unlocks a pair of row tiles.
  - SP ring order: weight half A, half B (2KB-line vd-halves of
    (Wv | WprojT) so the fold's first accumulation starts on A's
    semaphore), x chunks 0, 2, 3.  Act ring: x chunk 1 first (cold
    start absorbed mid-stream), bias (1 descriptor).
  - Fold W2T = Wv @ WprojT, f32r end-to-end; per tile the PSUM drain is
    a DVE tensor_add of the PE-broadcast bias block, emitting bf16
    (halves output bytes; host only zero-extends).
  - Outputs: 2-tile DMAs alternating rings as tiles drain; tile 6
    single; tile 7 split into two 64-partition DMAs, one per ring (half
    the trigger cost on the critical tail).

The host only moves bytes: it transposes/packs x and the weights and
unpermutes/widens the per-core output blocks (layout + zero-extension
only, no arithmetic).
"""

import os
import sys

import numpy as np

if "/opt/trn_rl_repo" not in sys.path:
    sys.path.insert(0, "/opt/trn_rl_repo")

B, N, C = 2, 4096, 256
ROWS = B * N              # 8192
NCORES = 8
RPC = ROWS // NCORES      # 1024 rows per core
NT = RPC // 128           # 8 row-tiles of 128 per core
NCHUNK = 4                # x column chunks per core (256 cols each)
CL = RPC // NCHUNK        # 256 cols per chunk

NWARM = int(os.environ.get("K_NWARM", "6"))    # PE clock-ramp matmul pairs
NWARM2 = int(os.environ.get("K_NWARM2", "3"))  # post-fold ramp filler pairs

_cache = {}


def _build():
    """Build + compile the per-core Bass program (same program, SPMD)."""
    import concourse.bacc as bacc
    import concourse.bass as bass
    import concourse.mybir as mybir
    import concourse.tile as tile

    f32 = mybir.dt.float32
    mm_dt = mybir.dt.float32r
    out_dt = mybir.dt.bfloat16

    nc = bacc.Bacc(
        "TRN2",
        target_bir_lowering=False,
        debug=False,
        num_devices=NCORES,
    )

    # All matmul inputs are typed f32r in DRAM too: the BIR verifier
    # requires every producer feeding an FP32r matmult to emit f32r, and
    # a DMA from an f32r DRAM tensor satisfies it (bytes are plain fp32).
    # xt2[p, j, kc, n] = x^T[kc*128 + p, j*256 + n]: chunk j is one
    # contiguous 2KB line per partition.
    xt2_d = nc.dram_tensor("xt2", [128, NCHUNK, 2, CL], mm_dt, kind="ExternalInput")
    # wb[p, 0]=Wv[p], [p,1]=WprojT[p], [p,2]=Wv[128+p], [p,3]=WprojT[128+p]
    wb_d = nc.dram_tensor("wb", [128, 4, C], mm_dt, kind="ExternalInput")
    b_d = nc.dram_tensor("b", [1, C], f32, kind="ExternalInput")
    # output laid out [p, t, m] so multi-tile DMAs get fat contiguous lines;
    # the host undoes the (t p) permutation
    out_d = nc.dram_tensor("out", [128, NT * C], out_dt, kind="ExternalOutput")

    xt2 = xt2_d.ap()
    wb = wb_d.ap()
    b = b_d.ap()
    out = out_d.ap()

    with tile.TileContext(nc) as tc:
        with (
            tc.tile_pool(name="const", bufs=1) as cp,
            tc.tile_pool(name="psw", bufs=3, space="PSUM") as psw,
            tc.tile_pool(name="pso", bufs=5, space="PSUM") as pso,
        ):
            # ---- weights first on SP as two 2KB-line DMAs: the first
            # half's semaphore starts the fold before the second half's
            # data lands ----
            wbA_sb = cp.tile([128, 2, C], mm_dt, tag="wbA")
            wbB_sb = cp.tile([128, 2, C], mm_dt, tag="wbB")
            nc.sync.dma_start(out=wbA_sb, in_=wb[:, 0:2, :])
            nc.sync.dma_start(out=wbB_sb, in_=wb[:, 2:4, :])

            # bias: ONE descriptor leading the Act ring (wakes the ring)
            bias_sb = cp.tile([1, C], f32)
            nc.scalar.dma_start(out=bias_sb, in_=b)
            ones_sb = cp.tile([1, 128], f32)
            nc.vector.memset(ones_sb, 1.0)

            # ---- x chunks: chunk 1 on Act (its ~2.5us cold start is
            # hidden: that data isn't needed until mid-stream); the rest
            # on SP in priority order ----
            xs = []
            for j in range(NCHUNK):
                xs.append(cp.tile([128, 2, CL], mm_dt, name=f"xchunk{j}",
                                  tag=f"xchunk{j}"))
            nc.scalar.dma_start(out=xs[1], in_=xt2[:, 1])
            nc.sync.dma_start(out=xs[0], in_=xt2[:, 0])
            nc.sync.dma_start(out=xs[2], in_=xt2[:, 2])
            nc.sync.dma_start(out=xs[3], in_=xt2[:, 3])

            # ---- PE warmup: sized so the burst ends right as the first
            # weight half's semaphore lands; keeps the PE's DVFS ramp
            # going without delaying the fold ----
            warm_sb = cp.tile([128, 128], f32)
            nc.vector.memset(warm_sb, 0.0)
            if NWARM:
                warm_ps = psw.tile([128, C], f32, tag="w")
                for _ in range(NWARM):
                    nc.tensor.matmul(
                        warm_ps[:, 0:128], warm_sb, warm_sb,
                        start=True, stop=True,
                    )

            # ---- fold W2T[k, p] = sum_vd Wv[vd, k] * WprojT[vd, p] ----
            # (f32r consumers, so the PSUM->SBUF copy emits f32r)
            w2t_sb = cp.tile([128, 2, C], mm_dt)  # [p(k), kc, pcol]
            ps_f = [psw.tile([128, C], f32, name=f"psf{i}", tag="w")
                    for i in range(2)]
            for kc in range(2):
                nc.tensor.matmul(
                    ps_f[kc],
                    wbA_sb[:, 0, kc * 128:(kc + 1) * 128],
                    wbA_sb[:, 1, :],
                    start=True, stop=False,
                )
            for kc in range(2):
                nc.tensor.matmul(
                    ps_f[kc],
                    wbB_sb[:, 0, kc * 128:(kc + 1) * 128],
                    wbB_sb[:, 1, :],
                    start=False, stop=True,
                )
                nc.vector.tensor_copy(w2t_sb[:, kc, :], ps_f[kc])

            # ---- post-fold ramp filler: keeps the PE's DVFS credit
            # accumulating across the fold -> first-x-chunk gap ----
            if NWARM2:
                warm_ps2 = psw.tile([128, C], f32, tag="w")
                for _ in range(NWARM2):
                    nc.tensor.matmul(
                        warm_ps2[:, 0:128], warm_sb, warm_sb,
                        start=True, stop=True,
                    )

            # broadcast bias across partitions via PE once; the DVE adds
            # it during each PSUM drain
            bias_bc = cp.tile([128, C], f32)
            ps_b = psw.tile([128, C], f32, tag="w")
            nc.tensor.matmul(ps_b, ones_sb, bias_sb, start=True, stop=True)
            nc.vector.tensor_copy(bias_bc, ps_b)

            # ---- main GEMM: out[n, p] = b[p] + sum_k xT[k, n]*W2T[k, p] ----
            ot_sb = cp.tile([128, NT, C], out_dt)
            for t in range(NT):
                j, off = t // 2, (t % 2) * 128
                ps = pso.tile([128, C], f32)
                nc.tensor.matmul(
                    ps, xs[j][:, 0, off:off + 128], w2t_sb[:, 0, :],
                    start=True, stop=False,
                )
                nc.tensor.matmul(
                    ps, xs[j][:, 1, off:off + 128], w2t_sb[:, 1, :],
                    start=False, stop=True,
                )
                nc.vector.tensor_add(ot_sb[:, t, :], ps, bias_bc)

                # output schedule: 2-tile chunks early on alternating
                # rings; the final tile split across both rings so its
                # trigger is half price on the critical tail
                if t == 1:
                    nc.sync.dma_start(out=out[:, 0:2 * C],
                                      in_=ot_sb[:, 0:2, :])
                elif t == 3:
                    nc.scalar.dma_start(out=out[:, 2 * C:4 * C],
                                        in_=ot_sb[:, 2:4, :])
                elif t == 5:
                    nc.sync.dma_start(out=out[:, 4 * C:6 * C],
                                      in_=ot_sb[:, 4:6, :])
                elif t == 6:
                    nc.scalar.dma_start(out=out[:, 6 * C:7 * C],
                                        in_=ot_sb[:, 6:7, :])
                elif t == 7:
                    nc.sync.dma_start(out=out[0:64, 7 * C:8 * C],
                                      in_=ot_sb[0:64, 7:8, :])
                    nc.scalar.dma_start(out=out[64:128, 7 * C:8 * C],
                                        in_=ot_sb[64:128, 7:8, :])

    nc.compile()
    return nc


def _pack_inputs(x, w_qkv, w_proj, b_proj):
    """Host-side layout marshaling only (no FLOPs)."""
    xT = np.ascontiguousarray(x.reshape(ROWS, C).T)          # [256, 8192]
    wv = w_qkv[2 * C:3 * C]                                  # [256, 256]
    wpt = w_proj.T                                           # [256, 256]
    wb = np.empty((128, 4, C), dtype=np.float32)
    wb[:, 0] = wv[0:128]
    wb[:, 1] = wpt[0:128]
    wb[:, 2] = wv[128:256]
    wb[:, 3] = wpt[128:256]
    wb = np.ascontiguousarray(wb)
    b2 = np.ascontiguousarray(b_proj.reshape(1, C))

    in_maps = []
    for c in range(NCORES):
        blk = xT[:, c * RPC:(c + 1) * RPC]                   # [256, 1024]
        # xt2[p, j, kc, n] = blk[kc*128 + p, j*CL + n]
        xt2 = np.ascontiguousarray(
            blk.reshape(2, 128, NCHUNK, CL).transpose(1, 2, 0, 3)
        )
        in_maps.append({"xt2": xt2, "wb": wb, "b": b2})
    return in_maps


def run_sharded(inputs, trace=False, trace_cores=None):
    """Shard inputs, run on the 8 NeuronCores, gather.  Returns
    (full_output, BassKernelResults)."""
    from concourse.bass_utils import run_bass_kernel_spmd

    x = np.ascontiguousarray(np.asarray(inputs["x"], dtype=np.float32))
    w_qkv = np.ascontiguousarray(np.asarray(inputs["w_qkv"], dtype=np.float32))
    w_proj = np.ascontiguousarray(np.asarray(inputs["w_proj"], dtype=np.float32))
    b_proj = np.ascontiguousarray(np.asarray(inputs["b_proj"], dtype=np.float32))

    if "nc" not in _cache:
        _cache["nc"] = _build()
    nc = _cache["nc"]

    in_maps = _pack_inputs(x, w_qkv, w_proj, b_proj)

    res = run_bass_kernel_spmd(
        nc,
        in_maps,
        core_ids=list(range(NCORES)),
        trace=trace,
        trace_cores=trace_cores,
    )
    # device emits [p, t, m]; undo the (t p) row permutation and widen
    # bf16 -> f32 (exact zero-extension)
    blocks = []
    for c in range(NCORES):
        arr = np.asarray(res.results[c]["out"]).reshape(128, NT, C)
        blocks.append(
            np.ascontiguousarray(arr.transpose(1, 0, 2)).reshape(RPC, C).astype(np.float32)
        )
    out = np.concatenate(blocks, axis=0)  # [8192, 256]
    return out.reshape(B, N, C), res


def kernel(x, w_qkv, w_proj, b_proj, temperature):
    out, _ = run_sharded(
        {"x": x, "w_qkv": w_qkv, "w_proj": w_proj, "b_proj": b_proj}
    )
    return out


# revision 18
# speedup vs baseline: 1.0792x; 1.0030x over previous
"""Trainium2 Bass kernel for nn_LocalitySelfAttention.

The module's attention scores get +1e9 added on the diagonal before the
softmax (torch's ``attn - diag(-1e9)``).  QK^T scores for randn inputs are
O(1), so every softmax row is an exact fp32 one-hot at the diagonal and
``attn @ v == v`` bit-exactly.  The whole module therefore reduces to

    out = x @ Wv.T @ w_proj.T + b_proj,      Wv = w_qkv[512:768]

which is a memory-bound GEMM.  The kernel shards the 8192 (B*N) rows across
the 8 NeuronCores (1024 rows each).  Each core:

  1. folds W2T[k,p] = sum_vd Wv[vd,k] * w_proj[p,vd] on the TensorEngine,
  2. computes out[n,p] = sum_k xT[k,n] * W2T[k,p] + b[p] as 8 PSUM tiles;
     the bias-add happens during the PSUM->SBUF copy on the DVE, emitting
     bf16 (the rounding is done on-device; the host only zero-extends
     bf16->f32, which is exact), halving both the copy time and the
     output HBM traffic.

All matmul operands are typed float32r end-to-end (DRAM + SBUF), which the
PE streams at half fp32's cycles-per-row; the bytes are plain fp32 and the
PSUM accumulation stays fp32 (rel err ~2e-3 vs 2e-2 tolerance).

Measured HW model this is built around:
  - exec_time = last-output-byte time + fixed overhead: the ~6us NEFF
    start is excluded by the profiler's first-useful-instruction window
    and an ~8.5us finalization tail is constant (the NRT postamble has
    each engine serially zeroing ~51 semaphores; Tensor is the straggler
    at ~144ns each), so everything aims at finishing the last output DMA
    byte early.  First DMA bytes land a fixed ~2.7us after the
    post-barrier triggers, and the 8-core input phase runs at the chip
    HBM roofline, so the input stream itself is the floor.
  - A DMA's completion semaphore fires when the SLOWEST of the 16 HW
    queues passes its descriptors; queue skew accumulates stochastically
    over the stream (HBM contention with the other 7 cores), so chunk
    semaphores bunch 1-2.5us after their data.  Schedules that gate the
    compute on finer chunk semaphores measured HIGHER-variance and
    WORSE-mean than this one, whose deterministic PE chain absorbs the
    noise.
  - dma_start runs at ~5ns/descriptor on the issuing engine and both
    HWDGE rings (SP, Act) feed the same 16 HW queues in descriptor
    ARRIVAL order, so transfers use >=2KB lines and issue order is
    arranged as: weights -> bias -> first x half (SP), then second x
    half (Act, gated on the weights' completion by a tiny Act read so
    it cannot starve the fold); outputs alternate across both rings.
  - x chunks each get their own SBUF tile (a shared buffer would
    serialize a chunk's DMA behind every reader of the previous chunk);
    the kc0/kc1 planes are separate DMAs, and the second 512 columns
    arrive as quarters so only two row-tiles of work remain after the
    final chunk's completion semaphore (which itself lands ~1-1.5us
    after the data: a DMA's 16 queue-shard completions spread out).
  - a 128-descriptor stride-0 broadcast DMA crawls (~75 B/ns) and blocks
    queue FIFOs, so the bias arrives as ONE descriptor and is broadcast
    across partitions by a one-time ones x bias matmul on the PE.
  - the final output tile is split into two 64-partition DMAs, one per
    ring, so its trigger (descriptor writing) is half price on the
    critical tail.

The host only moves bytes: it transposes x, packs the weight block, and
unpermutes/widens the per-core output blocks (layout + zero-extension
only, no arithmetic).
"""

import os
import sys

import numpy as np

if "/opt/trn_rl_repo" not in sys.path:
    sys.path.insert(0, "/opt/trn_rl_repo")

B, N, C = 2, 4096, 256
ROWS = B * N              # 8192
NCORES = 8
RPC = ROWS // NCORES      # 1024 rows per core
NT = RPC // 128           # 8 row-tiles of 128 per core

USE_F32R = os.environ.get("K_F32R", "1") == "1"
OUT_BF16 = os.environ.get("K_OBF16", "1") == "1"
NWARM = int(os.environ.get("K_NWARM", "0"))   # PE clock-ramp matmuls
SPLITLAST = os.environ.get("K_SPLITLAST", "1") == "1"

_cache = {}


def _build():
    """Build + compile the per-core Bass program (same program, SPMD)."""
    import concourse.bacc as bacc
    import concourse.bass as bass
    import concourse.mybir as mybir
    import concourse.tile as tile

    f32 = mybir.dt.float32
    mm_dt = mybir.dt.float32r if USE_F32R else f32
    out_dt = mybir.dt.bfloat16 if OUT_BF16 else f32

    nc = bacc.Bacc(
        "TRN2",
        target_bir_lowering=False,
        debug=False,
        num_devices=NCORES,
    )

    # All matmul inputs are typed f32r in DRAM too: the BIR verifier
    # requires every producer feeding an FP32r matmult to emit f32r, and
    # a DMA from an f32r DRAM tensor satisfies it (bytes are plain fp32).
    xt_d = nc.dram_tensor("xt", [C, RPC], mm_dt, kind="ExternalInput")
    wb_d = nc.dram_tensor("wb", [128, 4 * C], mm_dt, kind="ExternalInput")
    b_d = nc.dram_tensor("b", [C], f32, kind="ExternalInput")
    # output laid out [p, t, m] so multi-tile DMAs get fat contiguous lines;
    # the host undoes the (t p) permutation
    out_d = nc.dram_tensor("out", [128, NT * C], out_dt, kind="ExternalOutput")

    xt = xt_d.ap()
    wb = wb_d.ap()
    b = b_d.ap()
    out = out_d.ap()

    with tile.TileContext(nc) as tc:
        with (
            tc.tile_pool(name="const", bufs=1) as cp,
            tc.tile_pool(name="psw", bufs=3, space="PSUM") as psw,
            tc.tile_pool(name="pso", bufs=5, space="PSUM") as pso,
        ):
            # Both HWDGE rings (SP and Act) feed the SAME 16 HW queues in
            # descriptor-ARRIVAL order, so completion order is controlled
            # entirely by when each engine writes its descriptors.  Wanted
            # order: wb (fold) -> early x chunks -> late x chunks.

            # ---- weights first on SP: one DMA, 128 x 4KB lines ----
            # wb_sb[p, 0:2, k] = Wv[vdc*128+p, k]; [p, 2:4, q] = WprojT[vdc*128+p, q]
            wb_sb = cp.tile([128, 4, C], mm_dt)
            nc.sync.dma_start(out=wb_sb, in_=wb.rearrange("p (j k) -> p j k", j=4))

            # bias: ONE descriptor to a single partition (a 128-descriptor
            # stride-0 broadcast DMA crawls at ~75 B/ns and blocks every
            # queue FIFO behind it), then a one-time ones x bias matmul
            # broadcasts across partitions via the PE
            bias_sb = cp.tile([1, C], f32)
            nc.sync.dma_start(out=bias_sb, in_=b.rearrange("(o c) -> o c", o=1))
            ones_sb = cp.tile([1, 128], f32)
            nc.vector.memset(ones_sb, 1.0)

            # tiny Act-engine read of bias_sb: forces Act to wait until the
            # bias descriptor (queued right BEHIND all of wb's) completes
            # before issuing the late x chunks, so their descriptors arrive
            # after wb's and the fold is never starved behind x traffic.
            wgate = cp.tile([1, 16], f32)
            nc.scalar.copy(wgate, bias_sb[0:1, 0:16])

            # ---- x^T slice, k-major [k=256, n=1024], chunked by column
            # group x kc; first 512 columns on SP behind wb, second half on
            # Act behind the wb gate as quarters.  2KB lines measure FASTER
            # through the queues than a single 4KB-line p-major DMA. ----
            xt_v = xt.rearrange("(kc p) n -> p kc n", p=128)
            groups = [
                (nc.sync, 0, 512),
                (nc.scalar, 512, 256),
                (nc.scalar, 768, 256),
            ]
            tile_grp = [0, 0, 0, 0, 1, 1, 2, 2]   # row-tile -> chunk group
            xt_sbs = []      # [group][kc] -> tile [128, 1, col_len]
            for gi, (eng, c0, clen) in enumerate(groups):
                pair = []
                for kc in range(2):
                    xs = cp.tile([128, 1, clen], mm_dt, tag=f"xchunk{gi}_{kc}")
                    eng.dma_start(
                        out=xs,
                        in_=xt_v[:, kc:kc + 1, c0:c0 + clen],
                    )
                    pair.append(xs)
                xt_sbs.append(pair)

            # ---- PE warmup (off by default: the ramp helps the compute
            # phase but couples exec time to the stochastic semaphore
            # timing; the slow deterministic chain measures better) ----
            if NWARM:
                warm_sb = cp.tile([128, 128], f32)
                nc.vector.memset(warm_sb, 0.0)
                warm_ps = psw.tile([128, C], f32, tag="w")
                for _ in range(NWARM):
                    nc.tensor.matmul(
                        warm_ps[:, 0:128], warm_sb, warm_sb,
                        start=True, stop=True,
                    )

            # ones x bias -> all-partition bias row block (PE broadcast)
            bias_bc = cp.tile([128, C], f32)
            ps_b = psw.tile([128, C], f32, tag="w")
            nc.tensor.matmul(ps_b, ones_sb, bias_sb, start=True, stop=True)
            nc.vector.tensor_copy(bias_bc, ps_b)

            # ---- fold W2T[k, p] = sum_vd Wv[vd, k] * wpt[vd, p] ----
            # (f32r consumers, so the PSUM->SBUF copy emits f32r)
            w2t_sb = cp.tile([128, 2, C], mm_dt)  # [p(k), kc, pcol]
            for kc in range(2):
                ps = psw.tile([128, C], f32, tag="w")
                for vdc in range(2):
                    nc.tensor.matmul(
                        ps,
                        wb_sb[:, vdc, kc * 128:(kc + 1) * 128],
                        wb_sb[:, 2 + vdc, :],
                        start=(vdc == 0),
                        stop=(vdc == 1),
                    )
                nc.vector.tensor_copy(w2t_sb[:, kc, :], ps)

            # ---- main GEMM: out[n, p] = sum_k xT[k, n] * W2T[k, p] + b[p] ----
            # all 8 output tiles live in one contiguous SBUF block so output
            # DMAs can cover several tiles with one fat line per partition
            ot_sb = cp.tile([128, NT, C], out_dt)
            # output DMA schedule: 2-tile chunks early (their drain hides
            # under remaining compute), single-tile chunks for the last two
            # tiles on ALTERNATE rings so the final drain is minimal; the
            # very last tile is split across both rings by partition halves
            # (half-price trigger on the critical tail)
            out_sched = {1: (nc.scalar, 0), 3: (nc.sync, 2), 5: (nc.scalar, 4),
                         6: (nc.sync, 6)}
            for t in range(NT):
                gi = tile_grp[t]
                tc_off = t * 128 - groups[gi][1]
                ps = pso.tile([128, C], f32)
                xk0, xk1 = xt_sbs[gi]
                nc.tensor.matmul(
                    ps, xk0[:, 0, tc_off:tc_off + 128], w2t_sb[:, 0, :],
                    start=True, stop=False,
                )
                nc.tensor.matmul(
                    ps, xk1[:, 0, tc_off:tc_off + 128], w2t_sb[:, 1, :],
                    start=False, stop=True,
                )
                nc.vector.tensor_add(ot_sb[:, t, :], ps, bias_bc)
                if t in out_sched:
                    eng, t0 = out_sched[t]
                    eng.dma_start(
                        out=out[:, t0 * C:(t + 1) * C],
                        in_=ot_sb[:, t0:t + 1, :],
                    )
                elif t == 7:
                    if SPLITLAST:
                        nc.scalar.dma_start(out=out[0:64, 7 * C:8 * C],
                                            in_=ot_sb[0:64, 7:8, :])
                        nc.sync.dma_start(out=out[64:128, 7 * C:8 * C],
                                          in_=ot_sb[64:128, 7:8, :])
                    else:
                        nc.scalar.dma_start(out=out[:, 7 * C:8 * C],
                                            in_=ot_sb[:, 7:8, :])

    nc.compile()
    return nc


def run_sharded(inputs, trace=False, trace_cores=None):
    """Shard inputs, run on the 8 NeuronCores, gather.  Returns
    (full_output, BassKernelResults)."""
    from concourse.bass_utils import run_bass_kernel_spmd

    x = np.ascontiguousarray(np.asarray(inputs["x"], dtype=np.float32))
    w_qkv = np.ascontiguousarray(np.asarray(inputs["w_qkv"], dtype=np.float32))
    w_proj = np.ascontiguousarray(np.asarray(inputs["w_proj"], dtype=np.float32))
    b_proj = np.ascontiguousarray(np.asarray(inputs["b_proj"], dtype=np.float32))

    if "nc" not in _cache:
        _cache["nc"] = _build()
    nc = _cache["nc"]

    # host-side layout marshaling only (no FLOPs)
    xT = np.ascontiguousarray(x.reshape(ROWS, C).T)          # [256, 8192]
    wv = w_qkv[2 * C:3 * C]                                  # [256, 256]
    wpt = w_proj.T                                           # [256, 256]
    # pack wv + wpt p-major: wb[p, j, :] for j in (wv kc0, wv kc1, wpt 0, wpt 1)
    wb = np.empty((128, 4, C), dtype=np.float32)
    wb[:, 0] = wv[0:128]
    wb[:, 1] = wv[128:256]
    wb[:, 2] = wpt[0:128]
    wb[:, 3] = wpt[128:256]
    wb = np.ascontiguousarray(wb.reshape(128, 4 * C))

    in_maps = [
        {
            "xt": np.ascontiguousarray(xT[:, c * RPC:(c + 1) * RPC]),
            "wb": wb,
            "b": b_proj,
        }
        for c in range(NCORES)
    ]

    res = run_bass_kernel_spmd(
        nc,
        in_maps,
        core_ids=list(range(NCORES)),
        trace=trace,
        trace_cores=trace_cores,
    )
    # device emits [p, t, m]; undo the (t p) row permutation and widen
    # bf16 -> f32 (exact zero-extension)
    blocks = []
    for c in range(NCORES):
        arr = np.asarray(res.results[c]["out"]).reshape(128, NT, C)
        blocks.append(
            np.ascontiguousarray(arr.transpose(1, 0, 2)).reshape(RPC, C).astype(np.float32)
        )
    out = np.concatenate(blocks, axis=0)  # [8192, 256]
    return out.reshape(B, N, C), res


def kernel(x, w_qkv, w_proj, b_proj, temperature):
    out, _ = run_sharded(
        {"x": x, "w_qkv": w_qkv, "w_proj": w_proj, "b_proj": b_proj}
    )
    return out
